# revision 1
# baseline (speedup 1.0000x reference)
"""GIN message-passing kernel for 8 TRN2 NeuronCores.

Strategy (per sharding hint): nodes are sharded across 8 cores (6272 slots
each, 50176 padded total). Edges are partitioned by destination node so each
core's segment-sum is local; source-node features are gathered by row index
(dma_gather) from a replicated full table (x for layer 1, an AllGather'ed h1
for layer 2). MLP weights are replicated.

Per output tile of 128 nodes, in-edges (plus one self-edge per node) are
packed into 128-edge chunks. Each chunk is gathered as G [128 edges, 128
feats] and accumulated into a feature-major PSUM tile via
agg[f, i] += G.T @ M, where M[e, i] = (dst_off[e] == i) is a one-hot built
on-device with a single broadcast is_equal. dma_gather indices are int16, so
gathers are split at row 32768 (lo/hi base).
"""
import os
import warnings

warnings.filterwarnings("ignore")

import numpy as np

N = 50000
E = 800000
F = 128
H = 128
C = 40
BN_EPS = 1e-5
NCORES = 8
P = 128
NPC = 6272           # node slots per core
NT = NPC // P        # 49 tiles per core
NPAD = NCORES * NPC  # 50176
LOSPLIT = 32768      # int16 gather index limit


# ----------------------------------------------------------------- host prep

def _assign_nodes(deg):
    """Greedy balanced assignment of nodes to (core, tile, slot).

    Returns gid_of_orig[N]: global slot id = c*NPC + t*P + s, balancing total
    degree per core and per tile so per-tile chunk counts are uniform.
    """
    order = np.argsort(-deg, kind="stable")
    core_load = np.zeros(NCORES, np.int64)
    core_cnt = np.zeros(NCORES, np.int64)
    node_core = np.empty(N, np.int32)
    for n in order:
        c = -1
        best = None
        for cc in range(NCORES):
            if core_cnt[cc] >= NPC:
                continue
            if best is None or core_load[cc] < best:
                best = core_load[cc]
                c = cc
        node_core[n] = c
        core_load[c] += deg[n]
        core_cnt[c] += 1

    gid_of_orig = np.empty(N, np.int64)
    for c in range(NCORES):
        nodes = order[node_core[order] == c]
        tile_load = np.zeros(NT, np.int64)
        tile_cnt = np.zeros(NT, np.int64)
        tl = np.empty(len(nodes), np.int32)
        # nodes are already degree-sorted desc; greedy least-loaded tile
        for i, n in enumerate(nodes):
            avail = tile_cnt < P
            t = np.where(avail, tile_load, np.iinfo(np.int64).max).argmin()
            tl[i] = t
            tile_load[t] += deg[n]
            tile_cnt[t] += 1
        slot = np.zeros(NT, np.int64)
        for i, n in enumerate(nodes):
            t = tl[i]
            gid_of_orig[n] = c * NPC + t * P + slot[t]
            slot[t] += 1
    return gid_of_orig


def _wrap_idx(idx):
    """[n] int -> [128, n//16] int16: idx i at [i%16, i//16], replicated x8."""
    n = len(idx)
    w = np.asarray(idx, np.int16).reshape(n // 16, 16).T
    return np.tile(w, (8, 1))


def _pack_edges(src_gid, dst_gid, rowmap, rows_total):
    """Partition edges by (core, tile), split lo/hi by gathered row id, pad to
    128-multiples, and build per-core packed idx (int16 wrapped) and dstoff
    (f32) arrays plus the per-tile chunk-count lists.

    rowmap: maps src gid -> row index in the gather table (identity for x).
    Returns (CH_LO[t], CH_HI[t], idx_pack[c], off_pack[c]).
    """
    rows = rowmap[src_gid]
    core = dst_gid // NPC
    tile = (dst_gid % NPC) // P
    off = dst_gid % P
    is_lo = rows < LOSPLIT

    # bucket edges per (core, tile, half)
    lists = [[None] * NT for _ in range(NCORES)]
    key = (core * NT + tile).astype(np.int64)
    order = np.argsort(key, kind="stable")
    rows_s, off_s, lo_s, key_s = rows[order], off[order], is_lo[order], key[order]
    bounds = np.searchsorted(key_s, np.arange(NCORES * NT + 1))
    for c in range(NCORES):
        for t in range(NT):
            b0, b1 = bounds[c * NT + t], bounds[c * NT + t + 1]
            m = lo_s[b0:b1]
            lists[c][t] = (
                (rows_s[b0:b1][m], off_s[b0:b1][m]),
                (rows_s[b0:b1][~m] - LOSPLIT, off_s[b0:b1][~m]),
            )

    CH_LO = np.zeros(NT, np.int64)
    CH_HI = np.zeros(NT, np.int64)
    for c in range(NCORES):
        for t in range(NT):
            lo, hi = lists[c][t]
            CH_LO[t] = max(CH_LO[t], -(-len(lo[0]) // P))
            CH_HI[t] = max(CH_HI[t], -(-len(hi[0]) // P))
    CH_LO = np.maximum(CH_LO, 1)
    CH_HI = np.maximum(CH_HI, 1)

    idx_pack, off_pack = [], []
    hi_rows = rows_total - LOSPLIT
    for c in range(NCORES):
        idx_cols, off_cols = [], []
        for t in range(NT):
            (lor, loo), (hir, hio) = lists[c][t]
            nlo, nhi = CH_LO[t] * P, CH_HI[t] * P
            li = np.zeros(nlo, np.int64)
            li[: len(lor)] = lor
            lf = np.full(nlo, P, np.float32)
            lf[: len(loo)] = loo
            hi_ = np.zeros(nhi, np.int64)
            hi_[: len(hir)] = hir
            hf = np.full(nhi, P, np.float32)
            hf[: len(hio)] = hio
            assert li.max(initial=0) < LOSPLIT and hi_.max(initial=0) < hi_rows
            idx_cols += [_wrap_idx(li), _wrap_idx(hi_)]
            # dstoff layout [128, CH]: chunk k, partition p = edge k*128+p
            off_cols += [
                lf.reshape(CH_LO[t], P).T.astype(np.float32),
                hf.reshape(CH_HI[t], P).T.astype(np.float32),
            ]
        idx_pack.append(np.ascontiguousarray(np.concatenate(idx_cols, axis=1)))
        off_pack.append(np.ascontiguousarray(np.concatenate(off_cols, axis=1)))
    return CH_LO, CH_HI, idx_pack, off_pack


def prepare(x, edge_index, W1a, bn_gamma, bn_beta, bn_mean, bn_var, W1b, W2a, W2b):
    x = np.asarray(x, np.float32)
    ei = np.asarray(edge_index, np.int64)
    src_o, dst_o = ei[0], ei[1]

    deg = np.bincount(dst_o, minlength=N).astype(np.int64) + 1  # + self edge
    gid_of_orig = _assign_nodes(deg)

    # self edges fold the "+h" term of GIN into the segment sum
    src_gid = np.concatenate([gid_of_orig[src_o], gid_of_orig])
    dst_gid = np.concatenate([gid_of_orig[dst_o], gid_of_orig])

    ident_map = np.arange(NPAD, dtype=np.int64)
    CH_LO, CH_HI, idx_pack, off_pack = _pack_edges(src_gid, dst_gid, ident_map, NPAD)

    x_pad = np.zeros((NPAD, F), np.float32)
    x_pad[gid_of_orig] = x

    scale = (np.asarray(bn_gamma) / np.sqrt(np.asarray(bn_var) + BN_EPS)).astype(
        np.float32
    )
    bias = (np.asarray(bn_beta) - np.asarray(bn_mean) * scale).astype(np.float32)

    consts = {
        "x_pad": x_pad,
        "W1aT": np.ascontiguousarray(np.asarray(W1a, np.float32).T),
        "W1bT": np.ascontiguousarray(np.asarray(W1b, np.float32).T),
        "W2aT": np.ascontiguousarray(np.asarray(W2a, np.float32).T),
        "W2bT": np.ascontiguousarray(np.asarray(W2b, np.float32).T),
        "bn_s": scale.reshape(H, 1),
        "bn_b": bias.reshape(H, 1),
        "iota": np.tile(np.arange(P, dtype=np.float32), (P, 1)),
    }
    in_maps = []
    for c in range(NCORES):
        m = dict(consts)
        m["idx_all"] = idx_pack[c]
        m["off_all"] = off_pack[c]
        in_maps.append(m)
    return in_maps, CH_LO, CH_HI, gid_of_orig


# -------------------------------------------------------------- bass program

def build(CH_LO, CH_HI, repeat=1, do_gather=True, do_compute=True, do_cc=True):
    import concourse.bacc as bacc
    import concourse.mybir as mybir
    import concourse.tile as tile
    from concourse.masks import make_identity

    nc = bacc.Bacc("TRN2", target_bir_lowering=False, debug=False, num_devices=NCORES)
    f32 = mybir.dt.float32

    S_TOT = int(8 * (CH_LO.sum() + CH_HI.sum()))
    CH_TOT = int(CH_LO.sum() + CH_HI.sum())
    CH_MAX = int((CH_LO + CH_HI).max())

    x_pad = nc.dram_tensor("x_pad", [NPAD, F], f32, kind="ExternalInput")
    idx_all = nc.dram_tensor("idx_all", [P, S_TOT], mybir.dt.int16, kind="ExternalInput")
    off_all = nc.dram_tensor("off_all", [P, CH_TOT], f32, kind="ExternalInput")
    W1aT = nc.dram_tensor("W1aT", [F, H], f32, kind="ExternalInput")
    W1bT = nc.dram_tensor("W1bT", [H, H], f32, kind="ExternalInput")
    W2aT = nc.dram_tensor("W2aT", [H, H], f32, kind="ExternalInput")
    W2bT = nc.dram_tensor("W2bT", [H, C], f32, kind="ExternalInput")
    bn_s = nc.dram_tensor("bn_s", [H, 1], f32, kind="ExternalInput")
    bn_b = nc.dram_tensor("bn_b", [H, 1], f32, kind="ExternalInput")
    iota = nc.dram_tensor("iota", [P, P], f32, kind="ExternalInput")
    outT = nc.dram_tensor("outT", [C, NPC], f32, kind="ExternalOutput")

    Relu = mybir.ActivationFunctionType.Relu

    with tile.TileContext(nc) as tc:
        with (
            tc.tile_pool(name="const", bufs=1) as cst,
            tc.tile_pool(name="gbuf", bufs=3) as gp,
            tc.tile_pool(name="mbuf", bufs=3) as mp,
            tc.tile_pool(name="small", bufs=3) as sp,
            tc.tile_pool(name="ps_agg", bufs=2, space="PSUM") as ps_agg,
            tc.tile_pool(name="ps_mm", bufs=2, space="PSUM") as ps_mm,
            tc.tile_pool(name="dram", bufs=1, space="DRAM") as dram,
        ):
            ident = cst.tile([P, P], f32)
            make_identity(nc, ident[:])
            iota_sb = cst.tile([P, P], f32)
            nc.sync.dma_start(out=iota_sb[:], in_=iota[:])
            w1a_sb = cst.tile([F, H], f32)
            nc.sync.dma_start(out=w1a_sb[:], in_=W1aT[:])
            w1b_sb = cst.tile([H, H], f32)
            nc.sync.dma_start(out=w1b_sb[:], in_=W1bT[:])
            w2a_sb = cst.tile([H, H], f32)
            nc.sync.dma_start(out=w2a_sb[:], in_=W2aT[:])
            w2b_sb = cst.tile([H, C], f32)
            nc.sync.dma_start(out=w2b_sb[:], in_=W2bT[:])
            bns_sb = cst.tile([H, 1], f32)
            nc.sync.dma_start(out=bns_sb[:], in_=bn_s[:])
            bnb_sb = cst.tile([H, 1], f32)
            nc.sync.dma_start(out=bnb_sb[:], in_=bn_b[:])
            idx_sb = cst.tile([P, S_TOT], mybir.dt.int16)
            nc.sync.dma_start(out=idx_sb[:], in_=idx_all[:])
            off_sb = cst.tile([P, CH_TOT], f32)
            nc.sync.dma_start(out=off_sb[:], in_=off_all[:])

            h1_slice = dram.tile([NPC, H], f32)
            h1_full = dram.tile([NPAD, H], f32)

            # column offsets per tile into idx_all / off_all
            icol = np.concatenate([[0], np.cumsum((CH_LO + CH_HI) * 8)])
            ocol = np.concatenate([[0], np.cumsum(CH_LO + CH_HI)])

            def aggregate(t, table):
                """Gather + segment-sum for tile t -> PSUM [F, P] feat-major."""
                chl, chh = int(CH_LO[t]), int(CH_HI[t])
                ch = chl + chh
                G = gp.tile([P, CH_MAX, F], f32, tag="G")
                ic = int(icol[t])
                if do_gather:
                    nc.gpsimd.dma_gather(
                        G[:, 0:chl, :], table[0:LOSPLIT, :],
                        idx_sb[:, ic:ic + chl * 8], chl * P, chl * P, F,
                        single_packet=False,
                    )
                    nc.gpsimd.dma_gather(
                        G[:, chl:ch, :], table[LOSPLIT:NPAD, :],
                        idx_sb[:, ic + chl * 8: ic + ch * 8], chh * P, chh * P, F,
                        single_packet=False,
                    )
                if not do_compute:
                    return None
                M = mp.tile([P, CH_MAX * P], f32, tag="M")
                oc = int(ocol[t])
                nc.vector.tensor_tensor(
                    out=M[:, : ch * P],
                    in0=off_sb[:, oc:oc + ch, None].to_broadcast([P, ch, P]),
                    in1=iota_sb[:, None, :].to_broadcast([P, ch, P]),
                    op=mybir.AluOpType.is_equal,
                )
                agg_ps = ps_agg.tile([F, P], f32, tag="agg")
                for k in range(ch):
                    nc.tensor.matmul(
                        out=agg_ps[:],
                        lhsT=G[:, k, :],
                        rhs=M[:, k * P:(k + 1) * P],
                        start=(k == 0),
                        stop=(k == ch - 1),
                    )
                agg_sb = sp.tile([F, P], f32, tag="agg_sb")
                nc.vector.tensor_copy(out=agg_sb[:], in_=agg_ps[:])
                return agg_sb

            # ---- layer 1 ----
            for _rep in range(repeat):
             for t in range(NT):
                agg_sb = aggregate(t, x_pad)
                if agg_sb is None:
                    continue
                h1a_ps = ps_mm.tile([H, P], f32, tag="mma")
                nc.tensor.matmul(out=h1a_ps[:], lhsT=w1a_sb[:], rhs=agg_sb[:],
                                 start=True, stop=True)
                h1a_sb = sp.tile([H, P], f32, tag="h1a")
                nc.scalar.activation(out=h1a_sb[:], in_=h1a_ps[:], func=Relu,
                                     bias=bnb_sb[:, :1], scale=bns_sb[:, :1])
                h1b_ps = ps_mm.tile([H, P], f32, tag="mmb")
                nc.tensor.matmul(out=h1b_ps[:], lhsT=w1b_sb[:], rhs=h1a_sb[:],
                                 start=True, stop=True)
                h1b_sb = sp.tile([H, P], f32, tag="h1b")
                nc.scalar.activation(out=h1b_sb[:], in_=h1b_ps[:], func=Relu)
                ht_ps = ps_agg.tile([P, H], f32, tag="trans")
                nc.tensor.transpose(out=ht_ps[:], in_=h1b_sb[:], identity=ident[:])
                ht_sb = sp.tile([P, H], f32, tag="ht")
                nc.vector.tensor_copy(out=ht_sb[:], in_=ht_ps[:])
                nc.sync.dma_start(out=h1_slice[t * P:(t + 1) * P, :], in_=ht_sb[:])

             # ---- all-gather h1 ----
             if do_cc:
              nc.gpsimd.collective_compute(
                "AllGather",
                mybir.AluOpType.bypass,
                replica_groups=[list(range(NCORES))],
                ins=[h1_slice.opt()],
                outs=[h1_full.opt()],
              )

             # ---- layer 2 ----
             for t in range(NT):
                agg_sb = aggregate(t, h1_full)
                if agg_sb is None:
                    continue
                h2_ps = ps_mm.tile([H, P], f32, tag="mma")
                nc.tensor.matmul(out=h2_ps[:], lhsT=w2a_sb[:], rhs=agg_sb[:],
                                 start=True, stop=True)
                h2_sb = sp.tile([H, P], f32, tag="h1a")
                nc.scalar.activation(out=h2_sb[:], in_=h2_ps[:], func=Relu)
                o_ps = ps_mm.tile([C, P], f32, tag="mmb")
                nc.tensor.matmul(out=o_ps[:], lhsT=w2b_sb[:], rhs=h2_sb[:],
                                 start=True, stop=True)
                o_sb = sp.tile([C, P], f32, tag="out")
                nc.scalar.activation(out=o_sb[:], in_=o_ps[:], func=Relu)
                nc.sync.dma_start(out=outT[:, t * P:(t + 1) * P], in_=o_sb[:])

    nc.compile()
    return nc


# ------------------------------------------------------------------- driver

_CACHE = {}


def kernel(x, edge_index, W1a, bn_gamma, bn_beta, bn_mean, bn_var, W1b, W2a, W2b,
           _trace=False):
    from concourse.bass_utils import run_bass_kernel_spmd

    in_maps, CH_LO, CH_HI, gid_of_orig = prepare(
        x, edge_index, W1a, bn_gamma, bn_beta, bn_mean, bn_var, W1b, W2a, W2b
    )
    key = (tuple(CH_LO), tuple(CH_HI))
    if key not in _CACHE:
        _CACHE[key] = build(CH_LO, CH_HI)
    nc = _CACHE[key]

    res = run_bass_kernel_spmd(nc, in_maps, core_ids=list(range(NCORES)))
    outT = np.concatenate([r["outT"] for r in res.results], axis=1)  # [C, NPAD]
    out = outT.T[gid_of_orig]  # [N, C]
    if _trace:
        kernel.last_results = res
    return np.ascontiguousarray(out.astype(np.float32))



# revision 4
# speedup vs baseline: 1.9259x; 1.9259x over previous
"""GIN message-passing kernel for 8 TRN2 NeuronCores.

Strategy: nodes sharded across 8 cores (6272 slots each, 50176 padded).
Edges partitioned by destination node so each core's segment-sum is local;
source features are fetched by row with gpsimd.dma_gather from a replicated
table (x for layer 1, AllGather'ed h1 for layer 2). MLP weights replicated.

Perf structure (vs the f32 single-queue baseline):
- dma_gather descriptor generation on the Pool engine is the bottleneck
  (~4.2 ns/row floor); gathers are spread over 4 SWDGE queues so the DMA
  rings drain faster than desc-gen (single queue is ~9.2 ns/row).
- Tables, gathered edge blocks G, and the one-hot dst matrix M are bf16:
  halves DMA payload, doubles DVE/PE rates. PSUM accumulation stays f32.
- The GIN self term is not gathered: per tile, x_tile (or h1_tile) is DMA'd
  contiguously and added into the aggregation PSUM via one matmul against
  the identity.
- Table rows are remapped so the AllGather can be split into two chunked
  collectives (rows 0..25599 = all cores' tiles 0..24, rest = tiles 25..48);
  the first AG overlaps the tail of layer 1. Both sub-tables are < 32768
  rows, so int16 gather indices need no extra lo/hi split.
"""
import os
import warnings

warnings.filterwarnings("ignore")

import numpy as np
import ml_dtypes

N = 50000
E = 800000
F = 128
H = 128
C = 40
BN_EPS = 1e-5
NCORES = 8
P = 128
NPC = 6272            # node slots per core
NT = NPC // P         # 49 tiles per core
NTA = 25              # tiles in AG chunk A
NTB = NT - NTA        # tiles in AG chunk B
ROWS_A = NCORES * NTA * P   # 25600 rows in table chunk A
ROWS_B = NCORES * NTB * P   # 24576 rows in table chunk B
NPAD = NCORES * NPC   # 50176


# ----------------------------------------------------------------- host prep

def _assign_nodes(deg):
    """Greedy balanced assignment of nodes to (core, tile, slot).

    Returns gid_of_orig[N]: global slot id = c*NPC + t*P + s, balancing total
    degree per core and per tile so per-tile chunk counts are uniform.
    """
    order = np.argsort(-deg, kind="stable")
    core_load = np.zeros(NCORES, np.int64)
    core_cnt = np.zeros(NCORES, np.int64)
    node_core = np.empty(N, np.int32)
    for n in order:
        c = -1
        best = None
        for cc in range(NCORES):
            if core_cnt[cc] >= NPC:
                continue
            if best is None or core_load[cc] < best:
                best = core_load[cc]
                c = cc
        node_core[n] = c
        core_load[c] += deg[n]
        core_cnt[c] += 1

    gid_of_orig = np.empty(N, np.int64)
    for c in range(NCORES):
        nodes = order[node_core[order] == c]
        tile_load = np.zeros(NT, np.int64)
        tile_cnt = np.zeros(NT, np.int64)
        tl = np.empty(len(nodes), np.int32)
        # nodes are already degree-sorted desc; greedy least-loaded tile
        for i, n in enumerate(nodes):
            avail = tile_cnt < P
            t = np.where(avail, tile_load, np.iinfo(np.int64).max).argmin()
            tl[i] = t
            tile_load[t] += deg[n]
            tile_cnt[t] += 1
        slot = np.zeros(NT, np.int64)
        for i, n in enumerate(nodes):
            t = tl[i]
            gid_of_orig[n] = c * NPC + t * P + slot[t]
            slot[t] += 1
    return gid_of_orig


def _row_of_gid(gid):
    """Map gid (c*NPC + t*P + s) -> remapped table row.

    Chunk A rows: c*NTA*P + t*P + s       for t <  NTA
    Chunk B rows: ROWS_A + c*NTB*P + (t-NTA)*P + s  otherwise
    """
    c = gid // NPC
    r = gid % NPC
    t = r // P
    s = r % P
    return np.where(
        t < NTA,
        c * (NTA * P) + t * P + s,
        ROWS_A + c * (NTB * P) + (t - NTA) * P + s,
    )


def _wrap_idx(idx):
    """[n] int -> [128, n//16] int16: idx i at [i%16, i//16], replicated x8."""
    n = len(idx)
    w = np.asarray(idx, np.int16).reshape(n // 16, 16).T
    return np.tile(w, (8, 1))


def _pack_edges(src_gid, dst_gid):
    """Partition edges by (core, tile), split A/B by remapped source row,
    pad to 128-multiples, and build per-core packed idx (int16 wrapped) and
    dstoff (bf16) arrays plus the per-tile chunk-count lists.

    Returns (CH_A[t], CH_B[t], idx_pack[c], off_pack[c]).
    """
    rows = _row_of_gid(src_gid)
    core = dst_gid // NPC
    tile = (dst_gid % NPC) // P
    off = dst_gid % P
    is_a = rows < ROWS_A

    lists = [[None] * NT for _ in range(NCORES)]
    key = (core * NT + tile).astype(np.int64)
    order = np.argsort(key, kind="stable")
    rows_s, off_s, a_s, key_s = rows[order], off[order], is_a[order], key[order]
    bounds = np.searchsorted(key_s, np.arange(NCORES * NT + 1))
    for c in range(NCORES):
        for t in range(NT):
            b0, b1 = bounds[c * NT + t], bounds[c * NT + t + 1]
            m = a_s[b0:b1]
            lists[c][t] = (
                (rows_s[b0:b1][m], off_s[b0:b1][m]),
                (rows_s[b0:b1][~m] - ROWS_A, off_s[b0:b1][~m]),
            )

    CH_A = np.zeros(NT, np.int64)
    CH_B = np.zeros(NT, np.int64)
    for c in range(NCORES):
        for t in range(NT):
            a, b = lists[c][t]
            CH_A[t] = max(CH_A[t], -(-len(a[0]) // P))
            CH_B[t] = max(CH_B[t], -(-len(b[0]) // P))
    CH_A = np.maximum(CH_A, 1)
    CH_B = np.maximum(CH_B, 1)

    idx_pack, off_pack = [], []
    for c in range(NCORES):
        idx_cols, off_cols = [], []
        for t in range(NT):
            (ar, ao), (br, bo) = lists[c][t]
            na, nb = CH_A[t] * P, CH_B[t] * P
            ai = np.zeros(na, np.int64)
            ai[: len(ar)] = ar
            af = np.full(na, P, np.float32)
            af[: len(ao)] = ao
            bi = np.zeros(nb, np.int64)
            bi[: len(br)] = br
            bf = np.full(nb, P, np.float32)
            bf[: len(bo)] = bo
            assert ai.max(initial=0) < ROWS_A and bi.max(initial=0) < ROWS_B
            idx_cols += [_wrap_idx(ai), _wrap_idx(bi)]
            # dstoff layout [128, CH]: chunk k, partition p = edge k*128+p
            off_cols += [
                af.reshape(CH_A[t], P).T.astype(ml_dtypes.bfloat16),
                bf.reshape(CH_B[t], P).T.astype(ml_dtypes.bfloat16),
            ]
        idx_pack.append(np.ascontiguousarray(np.concatenate(idx_cols, axis=1)))
        off_pack.append(np.ascontiguousarray(np.concatenate(off_cols, axis=1)))
    return CH_A, CH_B, idx_pack, off_pack


def prepare(x, edge_index, W1a, bn_gamma, bn_beta, bn_mean, bn_var, W1b, W2a, W2b):
    x = np.asarray(x, np.float32)
    ei = np.asarray(edge_index, np.int64)
    src_o, dst_o = ei[0], ei[1]

    deg = np.bincount(dst_o, minlength=N).astype(np.int64) + 1  # + self edge
    gid_of_orig = _assign_nodes(deg)

    src_gid = gid_of_orig[src_o]
    dst_gid = gid_of_orig[dst_o]
    CH_A, CH_B, idx_pack, off_pack = _pack_edges(src_gid, dst_gid)

    # x table in remapped row order, bf16
    x_rm = np.zeros((NPAD, F), np.float32)
    x_rm[_row_of_gid(gid_of_orig)] = x
    x_rm = x_rm.astype(ml_dtypes.bfloat16)

    scale = (np.asarray(bn_gamma) / np.sqrt(np.asarray(bn_var) + BN_EPS)).astype(
        np.float32
    )
    bias = (np.asarray(bn_beta) - np.asarray(bn_mean) * scale).astype(np.float32)

    consts = {
        "x_pad": x_rm,
        "W1aT": np.ascontiguousarray(np.asarray(W1a, np.float32).T),
        "W1bT": np.ascontiguousarray(np.asarray(W1b, np.float32).T),
        "W2aT": np.ascontiguousarray(np.asarray(W2a, np.float32).T),
        "W2bT": np.ascontiguousarray(np.asarray(W2b, np.float32).T),
        "bn_s": scale.reshape(H, 1),
        "bn_b": bias.reshape(H, 1),
        "iota": np.tile(np.arange(P, dtype=np.float32), (P, 1)).astype(
            ml_dtypes.bfloat16
        ),
    }
    in_maps = []
    for c in range(NCORES):
        m = dict(consts)
        m["idx_all"] = idx_pack[c]
        m["off_all"] = off_pack[c]
        # per-core contiguous self rows (remapped order), for the self-term
        lo = np.ascontiguousarray(x_rm[c * (NTA * P):(c + 1) * (NTA * P)])
        hi_base = ROWS_A + c * (NTB * P)
        hi = np.ascontiguousarray(x_rm[hi_base:hi_base + NTB * P])
        m["x_self"] = np.concatenate([lo, hi], axis=0)  # [NPC, F] tile order
        in_maps.append(m)
    return in_maps, CH_A, CH_B, gid_of_orig


# -------------------------------------------------------------- bass program

def build(CH_A, CH_B, do_gather=True, do_compute=True, do_cc=True):
    import concourse.bacc as bacc
    import concourse.mybir as mybir
    import concourse.tile as tile
    from concourse.masks import make_identity

    nc = bacc.Bacc("TRN2", target_bir_lowering=False, debug=False,
                   num_devices=NCORES, num_swdge_queues=4)
    f32 = mybir.dt.float32
    bf16 = mybir.dt.bfloat16

    S_TOT = int(8 * (CH_A.sum() + CH_B.sum()))
    CH_TOT = int(CH_A.sum() + CH_B.sum())
    CH_MAX = int((CH_A + CH_B).max())

    x_pad = nc.dram_tensor("x_pad", [NPAD, F], bf16, kind="ExternalInput")
    x_self = nc.dram_tensor("x_self", [NPC, F], bf16, kind="ExternalInput")
    idx_all = nc.dram_tensor("idx_all", [P, S_TOT], mybir.dt.int16, kind="ExternalInput")
    off_all = nc.dram_tensor("off_all", [P, CH_TOT], bf16, kind="ExternalInput")
    W1aT = nc.dram_tensor("W1aT", [F, H], f32, kind="ExternalInput")
    W1bT = nc.dram_tensor("W1bT", [H, H], f32, kind="ExternalInput")
    W2aT = nc.dram_tensor("W2aT", [H, H], f32, kind="ExternalInput")
    W2bT = nc.dram_tensor("W2bT", [H, C], f32, kind="ExternalInput")
    bn_s = nc.dram_tensor("bn_s", [H, 1], f32, kind="ExternalInput")
    bn_b = nc.dram_tensor("bn_b", [H, 1], f32, kind="ExternalInput")
    iota = nc.dram_tensor("iota", [P, P], bf16, kind="ExternalInput")
    outT = nc.dram_tensor("outT", [C, NPC], f32, kind="ExternalOutput")

    Relu = mybir.ActivationFunctionType.Relu

    # queue assignment: greedy least-loaded by descriptor count
    qload = [0] * 4

    def pick_q(ndesc):
        q = min(range(4), key=lambda i: qload[i])
        qload[q] += ndesc
        return q

    with tile.TileContext(nc) as tc:
        with (
            tc.tile_pool(name="const", bufs=1) as cst,
            tc.tile_pool(name="gbuf", bufs=4) as gp,
            tc.tile_pool(name="mbuf", bufs=4) as mp,
            tc.tile_pool(name="sbuf", bufs=4) as sp,
            tc.tile_pool(name="selfb", bufs=4) as selfp,
            tc.tile_pool(name="ps_agg", bufs=2, space="PSUM") as ps_agg,
            tc.tile_pool(name="ps_mm", bufs=2, space="PSUM") as ps_mm,
            tc.tile_pool(name="ps_tr", bufs=2, space="PSUM") as ps_tr,
            tc.tile_pool(name="dram", bufs=1, space="DRAM") as dram,
        ):
            ident = cst.tile([P, P], bf16)
            make_identity(nc, ident[:])
            iota_sb = cst.tile([P, P], bf16)
            nc.sync.dma_start(out=iota_sb[:], in_=iota[:])
            w1a_sb = cst.tile([F, H], f32)
            nc.sync.dma_start(out=w1a_sb[:], in_=W1aT[:])
            w1b_sb = cst.tile([H, H], f32)
            nc.sync.dma_start(out=w1b_sb[:], in_=W1bT[:])
            w2a_sb = cst.tile([H, H], f32)
            nc.sync.dma_start(out=w2a_sb[:], in_=W2aT[:])
            w2b_sb = cst.tile([H, C], f32)
            nc.sync.dma_start(out=w2b_sb[:], in_=W2bT[:])
            bns_sb = cst.tile([H, 1], f32)
            nc.sync.dma_start(out=bns_sb[:], in_=bn_s[:])
            bnb_sb = cst.tile([H, 1], f32)
            nc.sync.dma_start(out=bnb_sb[:], in_=bn_b[:])
            idx_sb = cst.tile([P, S_TOT], mybir.dt.int16)
            nc.sync.dma_start(out=idx_sb[:], in_=idx_all[:])
            off_sb = cst.tile([P, CH_TOT], bf16)
            nc.sync.dma_start(out=off_sb[:], in_=off_all[:])

            h1_slice_a = dram.tile([NTA * P, H], bf16)
            h1_slice_b = dram.tile([NTB * P, H], bf16)
            h1_full_a = dram.tile([ROWS_A, H], bf16)
            h1_full_b = dram.tile([ROWS_B, H], bf16)

            # column offsets per tile into idx_all / off_all
            icol = np.concatenate([[0], np.cumsum((CH_A + CH_B) * 8)])
            ocol = np.concatenate([[0], np.cumsum(CH_A + CH_B)])

            def aggregate(t, table_a, table_b, self_src):
                """Gather + segment-sum for tile t -> PSUM [F, P] feat-major,
                including the self term via identity matmul."""
                cha, chb = int(CH_A[t]), int(CH_B[t])
                ch = cha + chb
                agg_ps = ps_agg.tile([F, P], f32, tag="agg")
                # self term: lhsT = self rows [P, F], rhs = identity
                st = selfp.tile([P, F], bf16, tag="self")
                nc.sync.dma_start(out=st[:], in_=self_src[t * P:(t + 1) * P, :])
                if do_gather:
                    G = gp.tile([P, CH_MAX, F], bf16, tag="G")
                    ic = int(icol[t])
                    nc.gpsimd.dma_gather(
                        G[:, 0:cha, :], table_a[:, :],
                        idx_sb[:, ic:ic + cha * 8], cha * P, cha * P, F,
                        single_packet=False, queue_num=pick_q(cha * P),
                    )
                    nc.gpsimd.dma_gather(
                        G[:, cha:ch, :], table_b[:, :],
                        idx_sb[:, ic + cha * 8: ic + ch * 8], chb * P, chb * P, F,
                        single_packet=False, queue_num=pick_q(chb * P),
                    )
                if not do_compute:
                    return None
                M = mp.tile([P, CH_MAX * P], bf16, tag="M")
                oc = int(ocol[t])
                nc.vector.tensor_tensor(
                    out=M[:, : ch * P],
                    in0=off_sb[:, oc:oc + ch, None].to_broadcast([P, ch, P]),
                    in1=iota_sb[:, None, :].to_broadcast([P, ch, P]),
                    op=mybir.AluOpType.is_equal,
                )
                nc.tensor.matmul(
                    out=agg_ps[:], lhsT=st[:], rhs=ident[:],
                    start=True, stop=False,
                )
                if do_gather:
                    for k in range(ch):
                        nc.tensor.matmul(
                            out=agg_ps[:],
                            lhsT=G[:, k, :],
                            rhs=M[:, k * P:(k + 1) * P],
                            start=False,
                            stop=(k == ch - 1),
                        )
                else:
                    nc.tensor.matmul(
                        out=agg_ps[:], lhsT=st[:], rhs=ident[:],
                        start=False, stop=True,
                    )
                agg_sb = sp.tile([F, P], f32, tag="agg_sb")
                nc.vector.tensor_copy(out=agg_sb[:], in_=agg_ps[:])
                return agg_sb

            # ---- layer 1 ----
            for t in range(NT):
                agg_sb = aggregate(t, x_pad[0:ROWS_A, :], x_pad[ROWS_A:NPAD, :],
                                   x_self)
                if agg_sb is None:
                    continue
                h1a_ps = ps_mm.tile([H, P], f32, tag="mma")
                nc.tensor.matmul(out=h1a_ps[:], lhsT=w1a_sb[:], rhs=agg_sb[:],
                                 start=True, stop=True)
                h1a_sb = sp.tile([H, P], f32, tag="h1a")
                nc.scalar.activation(out=h1a_sb[:], in_=h1a_ps[:], func=Relu,
                                     bias=bnb_sb[:, :1], scale=bns_sb[:, :1])
                h1b_ps = ps_mm.tile([H, P], f32, tag="mmb")
                nc.tensor.matmul(out=h1b_ps[:], lhsT=w1b_sb[:], rhs=h1a_sb[:],
                                 start=True, stop=True)
                h1b_sb = sp.tile([H, P], bf16, tag="h1b")
                nc.scalar.activation(out=h1b_sb[:], in_=h1b_ps[:], func=Relu)
                ht_ps = ps_tr.tile([P, H], bf16, tag="trans")
                nc.tensor.transpose(out=ht_ps[:], in_=h1b_sb[:], identity=ident[:])
                ht_sb = sp.tile([P, H], bf16, tag="ht")
                nc.vector.tensor_copy(out=ht_sb[:], in_=ht_ps[:])
                if t < NTA:
                    nc.sync.dma_start(out=h1_slice_a[t * P:(t + 1) * P, :],
                                      in_=ht_sb[:])
                else:
                    tb = t - NTA
                    nc.sync.dma_start(out=h1_slice_b[tb * P:(tb + 1) * P, :],
                                      in_=ht_sb[:])
                if do_cc and t == NTA - 1:
                    # all of slice A written -> gather it while tiles 25..48 run
                    nc.gpsimd.collective_compute(
                        "AllGather",
                        mybir.AluOpType.bypass,
                        replica_groups=[list(range(NCORES))],
                        ins=[h1_slice_a.opt()],
                        outs=[h1_full_a.opt()],
                    )

            if do_cc:
                nc.gpsimd.collective_compute(
                    "AllGather",
                    mybir.AluOpType.bypass,
                    replica_groups=[list(range(NCORES))],
                    ins=[h1_slice_b.opt()],
                    outs=[h1_full_b.opt()],
                )

            # ---- layer 2 ----
            h1_self = dram.tile([NPC, H], bf16)
            # local self rows in tile order = this core's own h1 slices
            nc.sync.dma_start(out=h1_self[0:NTA * P, :], in_=h1_slice_a[:])
            nc.sync.dma_start(out=h1_self[NTA * P:NPC, :], in_=h1_slice_b[:])

            for t in range(NT):
                agg_sb = aggregate(t, h1_full_a, h1_full_b, h1_self)
                if agg_sb is None:
                    continue
                h2_ps = ps_mm.tile([H, P], f32, tag="mma")
                nc.tensor.matmul(out=h2_ps[:], lhsT=w2a_sb[:], rhs=agg_sb[:],
                                 start=True, stop=True)
                h2_sb = sp.tile([H, P], f32, tag="h1a")
                nc.scalar.activation(out=h2_sb[:], in_=h2_ps[:], func=Relu)
                o_ps = ps_mm.tile([C, P], f32, tag="mmb")
                nc.tensor.matmul(out=o_ps[:], lhsT=w2b_sb[:], rhs=h2_sb[:],
                                 start=True, stop=True)
                o_sb = sp.tile([C, P], f32, tag="out")
                nc.scalar.activation(out=o_sb[:], in_=o_ps[:], func=Relu)
                nc.sync.dma_start(out=outT[:, t * P:(t + 1) * P], in_=o_sb[:])

    nc.compile()
    return nc


# ------------------------------------------------------------------- driver

_CACHE = {}


def kernel(x, edge_index, W1a, bn_gamma, bn_beta, bn_mean, bn_var, W1b, W2a, W2b,
           _trace=False):
    from concourse.bass_utils import run_bass_kernel_spmd

    in_maps, CH_A, CH_B, gid_of_orig = prepare(
        x, edge_index, W1a, bn_gamma, bn_beta, bn_mean, bn_var, W1b, W2a, W2b
    )
    key = (tuple(CH_A), tuple(CH_B))
    if key not in _CACHE:
        _CACHE[key] = build(CH_A, CH_B)
    nc = _CACHE[key]

    res = run_bass_kernel_spmd(nc, in_maps, core_ids=list(range(NCORES)))
    outT = np.concatenate([r["outT"] for r in res.results], axis=1)  # [C, NPAD]
    out = outT.T[gid_of_orig]  # [N, C]
    if _trace:
        kernel.last_results = res
    return np.ascontiguousarray(out.astype(np.float32))
